# revision 31
# baseline (speedup 1.0000x reference)
"""Tensor-parallel attention kernel for Trainium2 (8 NeuronCores).

Problem: B=2, L=2048, DIM=1024, H=16 heads, HD=64 (QKV proj + RoPE + SDPA + out proj).

Sharding: tensor-parallel over heads — 2 heads per core. Each core:
  - computes q/k/v for its 2 heads feature-major (qT/kT [128, 4096] = [2*64 hd, B*L]),
    via f32r matmuls of w_qkv column-slices against xT,
  - applies RoPE in-place (partition-swap via SBUF-SBUF DMA + DVE mul/add),
  - flash-style attention per (batch, 512-query tile): S^T chunks [128 keys, 512 q]
    on PE (2 heads row-packed), exp on ACT (psum -> f32r SBUF, fused 1/sqrt(hd) scale),
    P^T @ V accumulated on PE with a ones-column appended to V for the softmax
    denominator (M=65), normalization via DVE with DMA-broadcast reciprocal,
  - output projection against its w_out row-slice -> partial [4096, 1024].
Host sums the 8 partials (the "all-reduce after out_proj").
"""
import numpy as np

import concourse.bass as bass
import concourse.tile as tile
from concourse import bacc, mybir

B, L, DIM, H, HD = 2, 2048, 1024, 16, 64
NCORES = 8
HPC = H // NCORES            # heads per core = 2
T = B * L                    # 4096 tokens
NT = T // 512                # 8 token tiles of 512
KC = DIM // 128              # 8 contraction chunks for qkv
CH = T // 128                # 32 key chunks of 128 (global)
CHB = L // 128               # 16 key chunks per batch
QT = L // 512                # 4 query tiles per batch
VW = 2 * HD + 2              # v-nat chunk width: [V_A(64) | ones | V_B(64) | ones] = 130

F32 = mybir.dt.float32
F32R = mybir.dt.float32r
BF16 = mybir.dt.float16  # fp16: same PE rate as bf16, 8x mantissa
AF = mybir.ActivationFunctionType

_CACHE = {}


def _build_nc(reps=1, phases=(1, 2), io_lite=False, loop_reps=0):
    nc = bacc.Bacc("TRN2", target_bir_lowering=False, debug=False)

    if io_lite:
        # timing-only build: big tensors are device-internal (no per-call
        # host transfer); tiny dummy external IO keeps the runner shape.
        dummy_in = nc.dram_tensor("dummy_in", [128, 128], F32, kind="ExternalInput")
        dummy_out = nc.dram_tensor("dummy_out", [128, 128], F32, kind="ExternalOutput")
        xT_d = nc.dram_tensor("xT", [DIM, T], BF16)
        wqkv_d = nc.dram_tensor("wqkv", [128, KC, 3, 128], BF16)
        wout_d = nc.dram_tensor("wout", [128, 2, 512], BF16)
        cos_d = nc.dram_tensor("cosr", [64, L], F32)
        c2_d = nc.dram_tensor("c2r", [64, L], F32)
        perm_d = nc.dram_tensor("perm", [128, 128], BF16)
        out_d = nc.dram_tensor("out", [T, DIM], F32)
    else:
        xT_d = nc.dram_tensor("xT", [DIM, T], BF16, kind="ExternalInput")
        wqkv_d = nc.dram_tensor("wqkv", [128, KC, 3, 128], BF16, kind="ExternalInput")
        wout_d = nc.dram_tensor("wout", [128, 2, 512], BF16, kind="ExternalInput")
        cos_d = nc.dram_tensor("cosr", [64, L], F32, kind="ExternalInput")
        c2_d = nc.dram_tensor("c2r", [64, L], F32, kind="ExternalInput")
        perm_d = nc.dram_tensor("perm", [128, 128], BF16, kind="ExternalInput")
        out_d = nc.dram_tensor("out", [T, DIM], F32, kind="ExternalOutput")

    def rep2(dram_ap):
        # DRAM [64, L] read twice -> stream of 128 rows (partition-doubling)
        return bass.AP(tensor=dram_ap.tensor, offset=dram_ap.offset,
                       ap=[[0, 2]] + list(dram_ap.ap))

    with tile.TileContext(nc) as tc:
        from contextlib import ExitStack
        if io_lite:
            with tc.tile_pool(name="dummy", bufs=1) as dp:
                dt_ = dp.tile([128, 128], F32)
                nc.sync.dma_start(out=dt_[:], in_=dummy_in[:])
                nc.sync.dma_start(out=dummy_out[:], in_=dt_[:])
        loop_cm = tc.For_i(0, loop_reps, 1) if loop_reps else None
        if loop_cm is not None:
            loop_cm.__enter__()
        for _rep in range(reps):
          with ExitStack() as ctx:
            const = ctx.enter_context(tc.tile_pool(name="const", bufs=1))
            big = ctx.enter_context(tc.tile_pool(name="big", bufs=1))

            wqkv_sb = const.tile([128, KC, 3, 128], BF16)
            wout_sb = const.tile([128, 2, 512], BF16)
            cos_sb = const.tile([128, T], F32)
            c2_sb = const.tile([128, T], F32)
            ident = const.tile([128, 128], BF16)
            perm_sb = const.tile([128, 128], BF16)

            nc.sync.dma_start(out=wqkv_sb[:], in_=wqkv_d[:])
            nc.sync.dma_start(out=perm_sb[:], in_=perm_d[:])
            for b in range(B):
                sl = slice(b * L, (b + 1) * L)
                nc.sync.dma_start(out=cos_sb[:, sl], in_=rep2(cos_d[:]))
                nc.sync.dma_start(out=c2_sb[:, sl], in_=rep2(c2_d[:]))
            from concourse.masks import make_identity
            make_identity(nc, ident[:])

            qT = big.tile([128, T], BF16)
            kT = big.tile([128, T], BF16)
            vnat = big.tile([128, CH, VW], BF16)
            OT = big.tile([128, T], BF16)

            ones_sb = const.tile([128, CH], F32)
            nc.vector.memset(ones_sb[:], 1.0)
            nc.vector.tensor_copy(vnat[:, :, 64], ones_sb[:])
            nc.vector.tensor_copy(vnat[:, :, 129], ones_sb[:])

            # ---------------- Phase 1: QKV projection + RoPE + vT ----------------
            if 1 not in phases:
                continue
            with tc.tile_pool(name="qkv_ps", bufs=1, space="PSUM") as qkv_ps, \
                 tc.tile_pool(name="tr_ps", bufs=2, space="PSUM") as tr_ps, \
                 tc.tile_pool(name="rot_ps", bufs=2, space="PSUM") as rot_ps, \
                 tc.tile_pool(name="xt_pool", bufs=2) as xt_pool, \
                 tc.tile_pool(name="vt_pool", bufs=1) as vt_pool, \
                 tc.tile_pool(name="rope_pool", bufs=3) as rope_pool:

                vT = vt_pool.tile([128, T], BF16)
                for nt in range(NT):
                    csl = slice(nt * 512, (nt + 1) * 512)
                    psq = qkv_ps.tile([128, 512], F32, tag="psq")
                    psk = qkv_ps.tile([128, 512], F32, tag="psk")
                    psv = qkv_ps.tile([128, 512], F32, tag="psv")
                    xt = xt_pool.tile([128, KC, 512], BF16, tag="xt")
                    nc.sync.dma_start(
                        out=xt[:],
                        in_=xT_d[:, csl].rearrange("(c p) t -> p c t", p=128))
                    for kc in range(KC):
                        nc.tensor.matmul(psq[:], wqkv_sb[:, kc, 0, :], xt[:, kc, :],
                                         start=(kc == 0), stop=(kc == KC - 1))
                        nc.tensor.matmul(psk[:], wqkv_sb[:, kc, 1, :], xt[:, kc, :],
                                         start=(kc == 0), stop=(kc == KC - 1))
                        nc.tensor.matmul(psv[:], wqkv_sb[:, kc, 2, :], xt[:, kc, :],
                                         start=(kc == 0), stop=(kc == KC - 1))

                    # RoPE: dst = ps*cos + rot(ps)*c2, where rot is the signed
                    # half-swap per 64-block, computed on PE via a +-1 perm matmul
                    # (c2 here carries plain |sin| values; signs live in perm).
                    for ps, dst in ((psq, qT), (psk, kT)):
                        raw = rope_pool.tile([128, 512], BF16, tag="raw")
                        m1 = rope_pool.tile([128, 512], F32, tag="m1")
                        nc.vector.tensor_copy(raw[:], ps[:])
                        rps = rot_ps.tile([128, 512], F32, tag="rot")
                        nc.tensor.matmul(rps[:], perm_sb[:], raw[:])
                        m2 = rope_pool.tile([128, 512], F32, tag="m2")
                        nc.vector.tensor_mul(m1[:], ps[:], cos_sb[:, csl])
                        nc.vector.tensor_mul(m2[:], rps[:], c2_sb[:, csl])
                        nc.vector.tensor_add(dst[:, csl], m1[:], m2[:])

                    nc.vector.tensor_copy(vT[:, csl], psv[:])

                    # V transpose for this token tile: 4 chunks of 128 keys
                    for j in range(4):
                        c = nt * 4 + j
                        pst = tr_ps.tile([128, 128], BF16, tag="pst")
                        nc.tensor.transpose(
                            pst[:], vT[:, c * 128:(c + 1) * 128], ident[:])
                        nc.vector.tensor_copy(vnat[:, c, 0:64], pst[:, 0:64])
                        nc.vector.tensor_copy(vnat[:, c, 65:129], pst[:, 64:128])

            nc.sync.dma_start(out=wout_sb[:], in_=wout_d[:])

            # ---------------- Phase 2: attention + out projection ----------------
            if 2 not in phases:
                nc.sync.dma_start(out=out_d[0:128, :], in_=qT[:, 0:2048].bitcast(F32))
                continue
            with tc.tile_pool(name="s_ps", bufs=2, space="PSUM") as s_ps, \
                 tc.tile_pool(name="oa_ps", bufs=1, space="PSUM") as oa_ps, \
                 tc.tile_pool(name="ob_ps", bufs=1, space="PSUM") as ob_ps, \
                 tc.tile_pool(name="op_ps", bufs=2, space="PSUM") as op_ps, \
                 tc.tile_pool(name="pt_pool", bufs=4) as pt_pool, \
                 tc.tile_pool(name="d_pool", bufs=1) as d_pool, \
                 tc.tile_pool(name="rd_pool", bufs=2) as rd_pool, \
                 tc.tile_pool(name="o_pool", bufs=2) as o_pool, \
                 tc.tile_pool(name="st_pool", bufs=2) as st_pool:

                # d rows for all (b, qt, head): row 64 = denominators
                dall = d_pool.tile([65, 2, NT * 512 // 512, 512], F32)  # [65,2,8,512]

                def finalize(g):
                    # normalize OT slice for global qtile g and run its out-proj
                    qsl = slice(g * 512, (g + 1) * 512)
                    rdf = rd_pool.tile([128, 512], F32, tag="rdf")
                    nc.vector.reciprocal(dall[64:65, :, g, :], dall[64:65, :, g, :])
                    nc.gpsimd.dma_start(
                        out=dall[0:1, :, g, :], in_=dall[64:65, :, g, :])
                    rdb = rd_pool.tile([64, 512], F32, tag="rdb")
                    nc.gpsimd.partition_broadcast(
                        rdf[0:64, :], dall[0:1, 0, g, :], channels=64)
                    nc.gpsimd.partition_broadcast(
                        rdb[:], dall[0:1, 1, g, :], channels=64)
                    nc.gpsimd.dma_start(out=rdf[64:128, :], in_=rdb[:])
                    nc.vector.tensor_mul(OT[:, qsl], OT[:, qsl], rdf[:])
                    for j2 in range(2):
                        tch0 = g * 4 + j2 * 2
                        stg = st_pool.tile([128, 2, 1024], F32, tag="st")
                        for a in range(2):
                            tsl = slice((tch0 + a) * 128, (tch0 + a + 1) * 128)
                            for dj in range(2):
                                po = op_ps.tile([128, 512], F32, tag="po")
                                nc.tensor.matmul(
                                    po[:], OT[:, tsl], wout_sb[:, dj, :])
                                nc.vector.tensor_copy(
                                    stg[:, a, dj * 512:(dj + 1) * 512], po[:])
                        nc.sync.dma_start(
                            out=out_d[tch0 * 128:(tch0 + 2) * 128, :]
                                .rearrange("(a p) d -> p a d", p=128),
                            in_=stg[:])

                for b in range(B):
                    for qt in range(QT):
                        g = b * QT + qt
                        qsl = slice(g * 512, (g + 1) * 512)
                        if "nopv" not in phases:
                            oA = oa_ps.tile([65, 512], F32, tag="oA")
                            oB = ob_ps.tile([65, 512], F32, tag="oB")
                        for c in range(CHB):
                            cg = b * CHB + c
                            ksl = slice(cg * 128, (cg + 1) * 128)
                            first, last = (c == 0), (c == CHB - 1)
                            s2 = s_ps.tile([128, 2, 512], F32, tag="s")
                            nc.tensor.matmul(
                                s2[:, 0, :], kT[0:64, ksl], qT[0:64, qsl])
                            nc.tensor.matmul(
                                s2[:, 1, :], kT[64:128, ksl], qT[64:128, qsl])
                            pt = pt_pool.tile([128, 2, 512], BF16, tag="pt")
                            if "noexp" in phases:
                                nc.vector.tensor_copy(pt[:], s2[:])
                            else:
                                nc.scalar.activation(
                                    pt[:], s2[:], AF.Exp, scale=float(HD ** -0.5))
                            if "nopv" in phases:
                                continue
                            nc.tensor.matmul(
                                oA[:], vnat[:, cg, 0:65], pt[:, 0, :],
                                start=first, stop=last)
                            nc.tensor.matmul(
                                oB[:], vnat[:, cg, 65:130], pt[:, 1, :],
                                start=first, stop=last)

                        if "nopv" in phases:
                            continue
                        # evict unnormalized O + d rows; free psum banks fast
                        nc.vector.tensor_copy(OT[0:64, qsl], oA[0:64, :])
                        nc.vector.tensor_copy(dall[64:65, 0, g, :], oA[64:65, :])
                        otb = o_pool.tile([64, 512], BF16, tag="otb")
                        nc.vector.tensor_copy(otb[:], oB[0:64, :])
                        nc.vector.tensor_copy(dall[64:65, 1, g, :], oB[64:65, :])
                        nc.gpsimd.dma_start(out=OT[64:128, qsl], in_=otb[:])

                        if g > 0 and "nofin" not in phases:
                            finalize(g - 1)
                if "nofin" not in phases:
                    finalize(B * QT - 1)

        if loop_cm is not None:
            loop_cm.__exit__(None, None, None)

    nc.compile()
    return nc


def _host_prep(x, cos, sin, w_qkv, w_out):
    x = np.asarray(x, dtype=np.float32)
    cos = np.asarray(cos, dtype=np.float32)
    sin = np.asarray(sin, dtype=np.float32)
    w_qkv = np.asarray(w_qkv, dtype=np.float32)
    w_out = np.asarray(w_out, dtype=np.float32)

    xT = np.ascontiguousarray(x.reshape(T, DIM).T).astype(np.float16)
    cosr = np.ascontiguousarray(cos.T)                       # [64, L]
    c2 = np.ascontiguousarray(sin.T)                         # [64, L] plain sin
    # rot(ps)[p] = -ps[p+32] (p%64<32), +ps[p-32] (p%64>=32), as perm.T @ ps:
    # matmul computes out[m,n] = sum_k perm[k,m]*ps[k,n] -> perm[j,p] = coeff.
    perm = np.zeros((128, 128), dtype=np.float32)  # cast to bf16 below
    for blk in range(2):
        b0 = blk * 64
        for p in range(32):
            perm[b0 + p + 32, b0 + p] = -1.0      # out p<32 <- -ps[p+32]
            perm[b0 + p, b0 + p + 32] = 1.0       # out p>=32 <- +ps[p-32]

    in_maps = []
    for c in range(NCORES):
        h0 = c * HPC
        fs = slice(h0 * HD, h0 * HD + HPC * HD)              # 128 feature cols
        wc = np.concatenate(
            [w_qkv[:, 0 * H * HD:][:, fs],
             w_qkv[:, 1 * H * HD:][:, fs],
             w_qkv[:, 2 * H * HD:][:, fs]], axis=1)          # [1024, 384] = q|k|v
        # [kc*128+p, m*128+f] -> [p, kc, m, f]
        wq = np.ascontiguousarray(
            wc.reshape(KC, 128, 3, 128).transpose(1, 0, 2, 3)).astype(np.float16)
        wo = np.ascontiguousarray(
            w_out[fs, :].reshape(128, 2, 512)).astype(np.float16)
        in_maps.append({
            "xT": xT, "wqkv": wq, "wout": wo, "cosr": cosr, "c2r": c2,
            "perm": perm.astype(np.float16),
        })
    return in_maps


def _get_runner():
    if "runner" in _CACHE:
        return _CACHE["runner"]

    import jax
    from jax.sharding import Mesh, PartitionSpec
    from jax.experimental.shard_map import shard_map
    from concourse import bass2jax

    nc = _build_nc()
    bass2jax.install_neuronx_cc_hook()

    in_names = ["xT", "wqkv", "wout", "cosr", "c2r", "perm"]
    out_names = ["out"]
    out_avals = [jax.core.ShapedArray((T, DIM), np.float32)]
    bind_names = in_names + out_names
    if nc.partition_id_tensor is not None:
        bind_names = bind_names + [nc.partition_id_tensor.name]

    def _body(*args):
        operands = list(args)
        if nc.partition_id_tensor is not None:
            operands.append(bass2jax.partition_id_tensor())
        outs = bass2jax._bass_exec_p.bind(
            *operands,
            out_avals=tuple(out_avals),
            in_names=tuple(bind_names),
            out_names=tuple(out_names),
            lowering_input_output_aliases=(),
            sim_require_finite=True,
            sim_require_nnan=True,
            nc=nc,
        )
        return tuple(outs)

    devices = jax.devices()[:NCORES]
    mesh = Mesh(np.asarray(devices), ("core",))
    in_specs = (PartitionSpec("core"),) * (len(in_names) + 1)
    out_specs = (PartitionSpec("core"),)
    sharded = jax.jit(
        shard_map(_body, mesh=mesh, in_specs=in_specs, out_specs=out_specs,
                  check_rep=False),
        donate_argnums=(len(in_names),),
        keep_unused=True,
    )
    _CACHE["runner"] = (sharded, in_names)
    return _CACHE["runner"]


def device_inputs(in_maps):
    """Concatenate per-core input maps along axis 0 in runner arg order."""
    _, in_names = _get_runner()
    return [
        np.concatenate([np.asarray(m[name]) for m in in_maps], axis=0)
        for name in in_names
    ]


def run_sharded(in_maps):
    """Run the SPMD kernel; returns list of per-core output arrays [T, DIM]."""
    sharded, _ = _get_runner()
    concat_in = device_inputs(in_maps)
    zeros = np.zeros((NCORES * T, DIM), np.float32)
    (out,) = sharded(*concat_in, zeros)
    out = np.asarray(out).reshape(NCORES, T, DIM)
    return [out[c] for c in range(NCORES)]


def kernel(x, cos, sin, w_qkv, w_out):
    in_maps = _host_prep(x, cos, sin, w_qkv, w_out)
    parts = run_sharded(in_maps)
    full = parts[0].copy()
    for p in parts[1:]:
        full += p
    return full.reshape(B, L, DIM)


if __name__ == "__main__":
    rng = np.random.default_rng(0)
    x = rng.standard_normal((B, L, DIM), dtype=np.float32)
    import reference
    inputs = reference.setup_inputs()
    out = kernel(**{k: np.asarray(v) for k, v in inputs.items()})
    ref = np.asarray(reference.reference(**inputs))
    err = np.abs(out - ref)
    rel = np.sqrt((err ** 2).mean()) / np.sqrt((ref ** 2).mean())
    print("rms rel:", rel, "max abs:", err.max())


# revision 32
# speedup vs baseline: 1.1728x; 1.1728x over previous
"""Tensor-parallel attention kernel for Trainium2 (8 NeuronCores).

Problem: B=2, L=2048, DIM=1024, H=16 heads, HD=64 (QKV proj + RoPE + SDPA + out proj).

Sharding: tensor-parallel over heads — 2 heads per core. Each core:
  - computes q/k/v for its 2 heads feature-major (qT/kT [128, 4096] = [2*64 hd, B*L]),
    via f32r matmuls of w_qkv column-slices against xT,
  - applies RoPE in-place (partition-swap via SBUF-SBUF DMA + DVE mul/add),
  - flash-style attention per (batch, 512-query tile): S^T chunks [128 keys, 512 q]
    on PE (2 heads row-packed), exp on ACT (psum -> f32r SBUF, fused 1/sqrt(hd) scale),
    P^T @ V accumulated on PE with a ones-column appended to V for the softmax
    denominator (M=65), normalization via DVE with DMA-broadcast reciprocal,
  - output projection against its w_out row-slice -> partial [4096, 1024].
Host sums the 8 partials (the "all-reduce after out_proj").
"""
import numpy as np

import concourse.bass as bass
import concourse.tile as tile
from concourse import bacc, mybir

B, L, DIM, H, HD = 2, 2048, 1024, 16, 64
NCORES = 8
HPC = H // NCORES            # heads per core = 2
T = B * L                    # 4096 tokens
NT = T // 512                # 8 token tiles of 512
KC = DIM // 128              # 8 contraction chunks for qkv
CH = T // 128                # 32 key chunks of 128 (global)
CHB = L // 128               # 16 key chunks per batch
QT = L // 512                # 4 query tiles per batch
VW = 2 * HD + 2              # v-nat chunk width: [V_A(64) | ones | V_B(64) | ones] = 130

F32 = mybir.dt.float32
F32R = mybir.dt.float32r
BF16 = mybir.dt.float16  # fp16: same PE rate as bf16, 8x mantissa
AF = mybir.ActivationFunctionType

_CACHE = {}


def _build_nc(reps=1, phases=(1, 2), io_lite=False, loop_reps=0):
    nc = bacc.Bacc("TRN2", target_bir_lowering=False, debug=False)

    if io_lite:
        # timing-only build: big tensors are device-internal (no per-call
        # host transfer); tiny dummy external IO keeps the runner shape.
        dummy_in = nc.dram_tensor("dummy_in", [128, 128], F32, kind="ExternalInput")
        dummy_out = nc.dram_tensor("dummy_out", [128, 128], F32, kind="ExternalOutput")
        xT_d = nc.dram_tensor("xT", [DIM, T], BF16)
        wqkv_d = nc.dram_tensor("wqkv", [128, KC, 3, 128], BF16)
        wout_d = nc.dram_tensor("wout", [128, 2, 512], BF16)
        cos_d = nc.dram_tensor("cosr", [64, L], F32)
        c2_d = nc.dram_tensor("c2r", [64, L], F32)
        perm_d = nc.dram_tensor("perm", [128, 128], BF16)
        out_d = nc.dram_tensor("out", [T, DIM], BF16)
    else:
        xT_d = nc.dram_tensor("xT", [DIM, T], BF16, kind="ExternalInput")
        wqkv_d = nc.dram_tensor("wqkv", [128, KC, 3, 128], BF16, kind="ExternalInput")
        wout_d = nc.dram_tensor("wout", [128, 2, 512], BF16, kind="ExternalInput")
        cos_d = nc.dram_tensor("cosr", [64, L], F32, kind="ExternalInput")
        c2_d = nc.dram_tensor("c2r", [64, L], F32, kind="ExternalInput")
        perm_d = nc.dram_tensor("perm", [128, 128], BF16, kind="ExternalInput")
        out_d = nc.dram_tensor("out", [T, DIM], BF16, kind="ExternalOutput")

    def rep2(dram_ap):
        # DRAM [64, L] read twice -> stream of 128 rows (partition-doubling)
        return bass.AP(tensor=dram_ap.tensor, offset=dram_ap.offset,
                       ap=[[0, 2]] + list(dram_ap.ap))

    with tile.TileContext(nc) as tc:
        from contextlib import ExitStack
        if io_lite:
            with tc.tile_pool(name="dummy", bufs=1) as dp:
                dt_ = dp.tile([128, 128], F32)
                nc.sync.dma_start(out=dt_[:], in_=dummy_in[:])
                nc.sync.dma_start(out=dummy_out[:], in_=dt_[:])
        loop_cm = tc.For_i(0, loop_reps, 1) if loop_reps else None
        if loop_cm is not None:
            loop_cm.__enter__()
        for _rep in range(reps):
          with ExitStack() as ctx:
            const = ctx.enter_context(tc.tile_pool(name="const", bufs=1))
            big = ctx.enter_context(tc.tile_pool(name="big", bufs=1))

            wqkv_sb = const.tile([128, KC, 3, 128], BF16)
            wout_sb = const.tile([128, 2, 512], BF16)
            cos_sb = const.tile([128, T], F32)
            c2_sb = const.tile([128, T], F32)
            ident = const.tile([128, 128], BF16)
            perm_sb = const.tile([128, 128], BF16)

            nc.sync.dma_start(out=wqkv_sb[:], in_=wqkv_d[:])
            nc.sync.dma_start(out=perm_sb[:], in_=perm_d[:])
            for b in range(B):
                sl = slice(b * L, (b + 1) * L)
                nc.sync.dma_start(out=cos_sb[:, sl], in_=rep2(cos_d[:]))
                nc.sync.dma_start(out=c2_sb[:, sl], in_=rep2(c2_d[:]))
            from concourse.masks import make_identity
            make_identity(nc, ident[:])

            qT = big.tile([128, T], BF16)
            kT = big.tile([128, T], BF16)
            vnat = big.tile([128, CH, VW], BF16)
            OT = big.tile([128, T], BF16)

            ones_sb = const.tile([128, CH], F32)
            nc.vector.memset(ones_sb[:], 1.0)
            nc.vector.tensor_copy(vnat[:, :, 64], ones_sb[:])
            nc.vector.tensor_copy(vnat[:, :, 129], ones_sb[:])

            # ---------------- Phase 1: QKV projection + RoPE + vT ----------------
            if 1 not in phases:
                continue
            with tc.tile_pool(name="qkv_ps", bufs=2, space="PSUM") as qkv_ps, \
                 tc.tile_pool(name="tr_ps", bufs=1, space="PSUM") as tr_ps, \
                 tc.tile_pool(name="rot_ps", bufs=1, space="PSUM") as rot_ps, \
                 tc.tile_pool(name="xt_pool", bufs=2) as xt_pool, \
                 tc.tile_pool(name="vt_pool", bufs=1) as vt_pool, \
                 tc.tile_pool(name="rope_pool", bufs=3) as rope_pool:

                vT = vt_pool.tile([128, T], BF16)
                for nt in range(NT):
                    csl = slice(nt * 512, (nt + 1) * 512)
                    psq = qkv_ps.tile([128, 512], F32, tag="psq")
                    psk = qkv_ps.tile([128, 512], F32, tag="psk")
                    psv = qkv_ps.tile([128, 512], F32, tag="psv")
                    xt = xt_pool.tile([128, KC, 512], BF16, tag="xt")
                    nc.sync.dma_start(
                        out=xt[:],
                        in_=xT_d[:, csl].rearrange("(c p) t -> p c t", p=128))
                    for kc in range(KC):
                        nc.tensor.matmul(psq[:], wqkv_sb[:, kc, 0, :], xt[:, kc, :],
                                         start=(kc == 0), stop=(kc == KC - 1))
                        nc.tensor.matmul(psk[:], wqkv_sb[:, kc, 1, :], xt[:, kc, :],
                                         start=(kc == 0), stop=(kc == KC - 1))
                        nc.tensor.matmul(psv[:], wqkv_sb[:, kc, 2, :], xt[:, kc, :],
                                         start=(kc == 0), stop=(kc == KC - 1))

                    # RoPE: dst = ps*cos + rot(ps)*c2, where rot is the signed
                    # half-swap per 64-block, computed on PE via a +-1 perm matmul
                    # (c2 here carries plain |sin| values; signs live in perm).
                    for ps, dst in ((psq, qT), (psk, kT)):
                        raw = rope_pool.tile([128, 512], BF16, tag="raw")
                        m1 = rope_pool.tile([128, 512], F32, tag="m1")
                        nc.vector.tensor_copy(raw[:], ps[:])
                        rps = rot_ps.tile([128, 512], F32, tag="rot")
                        nc.tensor.matmul(rps[:], perm_sb[:], raw[:])
                        m2 = rope_pool.tile([128, 512], F32, tag="m2")
                        nc.vector.tensor_mul(m1[:], ps[:], cos_sb[:, csl])
                        nc.vector.tensor_mul(m2[:], rps[:], c2_sb[:, csl])
                        nc.vector.tensor_add(dst[:, csl], m1[:], m2[:])

                    nc.vector.tensor_copy(vT[:, csl], psv[:])

                    # V transpose for this token tile: 4 chunks of 128 keys
                    for j in range(4):
                        c = nt * 4 + j
                        pst = tr_ps.tile([128, 128], BF16, tag="pst")
                        nc.tensor.transpose(
                            pst[:], vT[:, c * 128:(c + 1) * 128], ident[:])
                        nc.vector.tensor_copy(vnat[:, c, 0:64], pst[:, 0:64])
                        nc.vector.tensor_copy(vnat[:, c, 65:129], pst[:, 64:128])

            nc.sync.dma_start(out=wout_sb[:], in_=wout_d[:])

            # ---------------- Phase 2: attention + out projection ----------------
            if 2 not in phases:
                nc.sync.dma_start(out=out_d[0:128, :], in_=qT[:, 0:2048].bitcast(F32))
                continue
            with tc.tile_pool(name="s_ps", bufs=2, space="PSUM") as s_ps, \
                 tc.tile_pool(name="oa_ps", bufs=1, space="PSUM") as oa_ps, \
                 tc.tile_pool(name="ob_ps", bufs=1, space="PSUM") as ob_ps, \
                 tc.tile_pool(name="op_ps", bufs=2, space="PSUM") as op_ps, \
                 tc.tile_pool(name="pt_pool", bufs=4) as pt_pool, \
                 tc.tile_pool(name="d_pool", bufs=1) as d_pool, \
                 tc.tile_pool(name="rd_pool", bufs=2) as rd_pool, \
                 tc.tile_pool(name="o_pool", bufs=2) as o_pool, \
                 tc.tile_pool(name="st_pool", bufs=2) as st_pool:

                # d rows for all (b, qt, head): row 64 = denominators
                dall = d_pool.tile([65, 2, NT * 512 // 512, 512], F32)  # [65,2,8,512]

                def finalize(g):
                    # normalize OT slice for global qtile g and run its out-proj
                    qsl = slice(g * 512, (g + 1) * 512)
                    rdf = rd_pool.tile([128, 512], F32, tag="rdf")
                    nc.vector.reciprocal(dall[64:65, :, g, :], dall[64:65, :, g, :])
                    nc.gpsimd.dma_start(
                        out=dall[0:1, :, g, :], in_=dall[64:65, :, g, :])
                    rdb = rd_pool.tile([64, 512], F32, tag="rdb")
                    nc.gpsimd.partition_broadcast(
                        rdf[0:64, :], dall[0:1, 0, g, :], channels=64)
                    nc.gpsimd.partition_broadcast(
                        rdb[:], dall[0:1, 1, g, :], channels=64)
                    nc.gpsimd.dma_start(out=rdf[64:128, :], in_=rdb[:])
                    nc.vector.tensor_mul(OT[:, qsl], OT[:, qsl], rdf[:])
                    for j2 in range(2):
                        tch0 = g * 4 + j2 * 2
                        stg = st_pool.tile([128, 2, 1024], BF16, tag="st")
                        for a in range(2):
                            tsl = slice((tch0 + a) * 128, (tch0 + a + 1) * 128)
                            for dj in range(2):
                                po = op_ps.tile([128, 512], F32, tag="po")
                                nc.tensor.matmul(
                                    po[:], OT[:, tsl], wout_sb[:, dj, :])
                                dst = stg[:, a, dj * 512:(dj + 1) * 512]
                                if dj == 0:
                                    nc.vector.tensor_copy(dst, po[:])
                                else:
                                    nc.scalar.copy(dst, po[:])
                        nc.sync.dma_start(
                            out=out_d[tch0 * 128:(tch0 + 2) * 128, :]
                                .rearrange("(a p) d -> p a d", p=128),
                            in_=stg[:])

                for b in range(B):
                    for qt in range(QT):
                        g = b * QT + qt
                        qsl = slice(g * 512, (g + 1) * 512)
                        if "nopv" not in phases:
                            oA = oa_ps.tile([65, 512], F32, tag="oA")
                            oB = ob_ps.tile([65, 512], F32, tag="oB")
                        for c in range(CHB):
                            cg = b * CHB + c
                            ksl = slice(cg * 128, (cg + 1) * 128)
                            first, last = (c == 0), (c == CHB - 1)
                            s2 = s_ps.tile([128, 2, 512], F32, tag="s")
                            nc.tensor.matmul(
                                s2[:, 0, :], kT[0:64, ksl], qT[0:64, qsl])
                            nc.tensor.matmul(
                                s2[:, 1, :], kT[64:128, ksl], qT[64:128, qsl])
                            pt = pt_pool.tile([128, 2, 512], BF16, tag="pt")
                            if "noexp" in phases:
                                nc.vector.tensor_copy(pt[:], s2[:])
                            else:
                                nc.scalar.activation(
                                    pt[:], s2[:], AF.Exp, scale=float(HD ** -0.5))
                            if "nopv" in phases:
                                continue
                            nc.tensor.matmul(
                                oA[:], vnat[:, cg, 0:65], pt[:, 0, :],
                                start=first, stop=last)
                            nc.tensor.matmul(
                                oB[:], vnat[:, cg, 65:130], pt[:, 1, :],
                                start=first, stop=last)

                        if "nopv" in phases:
                            continue
                        # evict unnormalized O + d rows; free psum banks fast
                        nc.vector.tensor_copy(OT[0:64, qsl], oA[0:64, :])
                        nc.vector.tensor_copy(dall[64:65, 0, g, :], oA[64:65, :])
                        otb = o_pool.tile([64, 512], BF16, tag="otb")
                        nc.vector.tensor_copy(otb[:], oB[0:64, :])
                        nc.vector.tensor_copy(dall[64:65, 1, g, :], oB[64:65, :])
                        nc.gpsimd.dma_start(out=OT[64:128, qsl], in_=otb[:])

                        if g > 0 and "nofin" not in phases:
                            finalize(g - 1)
                if "nofin" not in phases:
                    finalize(B * QT - 1)

        if loop_cm is not None:
            loop_cm.__exit__(None, None, None)

    nc.compile()
    return nc


def _host_prep(x, cos, sin, w_qkv, w_out):
    x = np.asarray(x, dtype=np.float32)
    cos = np.asarray(cos, dtype=np.float32)
    sin = np.asarray(sin, dtype=np.float32)
    w_qkv = np.asarray(w_qkv, dtype=np.float32)
    w_out = np.asarray(w_out, dtype=np.float32)

    xT = np.ascontiguousarray(x.reshape(T, DIM).T).astype(np.float16)
    cosr = np.ascontiguousarray(cos.T)                       # [64, L]
    c2 = np.ascontiguousarray(sin.T)                         # [64, L] plain sin
    # rot(ps)[p] = -ps[p+32] (p%64<32), +ps[p-32] (p%64>=32), as perm.T @ ps:
    # matmul computes out[m,n] = sum_k perm[k,m]*ps[k,n] -> perm[j,p] = coeff.
    perm = np.zeros((128, 128), dtype=np.float32)  # cast to bf16 below
    for blk in range(2):
        b0 = blk * 64
        for p in range(32):
            perm[b0 + p + 32, b0 + p] = -1.0      # out p<32 <- -ps[p+32]
            perm[b0 + p, b0 + p + 32] = 1.0       # out p>=32 <- +ps[p-32]

    in_maps = []
    for c in range(NCORES):
        h0 = c * HPC
        fs = slice(h0 * HD, h0 * HD + HPC * HD)              # 128 feature cols
        wc = np.concatenate(
            [w_qkv[:, 0 * H * HD:][:, fs],
             w_qkv[:, 1 * H * HD:][:, fs],
             w_qkv[:, 2 * H * HD:][:, fs]], axis=1)          # [1024, 384] = q|k|v
        # [kc*128+p, m*128+f] -> [p, kc, m, f]
        wq = np.ascontiguousarray(
            wc.reshape(KC, 128, 3, 128).transpose(1, 0, 2, 3)).astype(np.float16)
        wo = np.ascontiguousarray(
            w_out[fs, :].reshape(128, 2, 512)).astype(np.float16)
        in_maps.append({
            "xT": xT, "wqkv": wq, "wout": wo, "cosr": cosr, "c2r": c2,
            "perm": perm.astype(np.float16),
        })
    return in_maps


def _get_runner():
    if "runner" in _CACHE:
        return _CACHE["runner"]

    import jax
    from jax.sharding import Mesh, PartitionSpec
    from jax.experimental.shard_map import shard_map
    from concourse import bass2jax

    nc = _build_nc()
    bass2jax.install_neuronx_cc_hook()

    in_names = ["xT", "wqkv", "wout", "cosr", "c2r", "perm"]
    out_names = ["out"]
    out_avals = [jax.core.ShapedArray((T, DIM), np.float16)]
    bind_names = in_names + out_names
    if nc.partition_id_tensor is not None:
        bind_names = bind_names + [nc.partition_id_tensor.name]

    def _body(*args):
        operands = list(args)
        if nc.partition_id_tensor is not None:
            operands.append(bass2jax.partition_id_tensor())
        outs = bass2jax._bass_exec_p.bind(
            *operands,
            out_avals=tuple(out_avals),
            in_names=tuple(bind_names),
            out_names=tuple(out_names),
            lowering_input_output_aliases=(),
            sim_require_finite=True,
            sim_require_nnan=True,
            nc=nc,
        )
        return tuple(outs)

    devices = jax.devices()[:NCORES]
    mesh = Mesh(np.asarray(devices), ("core",))
    in_specs = (PartitionSpec("core"),) * (len(in_names) + 1)
    out_specs = (PartitionSpec("core"),)
    sharded = jax.jit(
        shard_map(_body, mesh=mesh, in_specs=in_specs, out_specs=out_specs,
                  check_rep=False),
        donate_argnums=(len(in_names),),
        keep_unused=True,
    )
    _CACHE["runner"] = (sharded, in_names)
    return _CACHE["runner"]


def device_inputs(in_maps):
    """Concatenate per-core input maps along axis 0 in runner arg order."""
    _, in_names = _get_runner()
    return [
        np.concatenate([np.asarray(m[name]) for m in in_maps], axis=0)
        for name in in_names
    ]


def run_sharded(in_maps):
    """Run the SPMD kernel; returns list of per-core output arrays [T, DIM]."""
    sharded, _ = _get_runner()
    concat_in = device_inputs(in_maps)
    zeros = np.zeros((NCORES * T, DIM), np.float16)
    (out,) = sharded(*concat_in, zeros)
    out = np.asarray(out).reshape(NCORES, T, DIM)
    return [out[c] for c in range(NCORES)]


def kernel(x, cos, sin, w_qkv, w_out):
    in_maps = _host_prep(x, cos, sin, w_qkv, w_out)
    parts = run_sharded(in_maps)
    full = parts[0].astype(np.float32)
    for p in parts[1:]:
        full += p.astype(np.float32)
    return full.reshape(B, L, DIM)


if __name__ == "__main__":
    rng = np.random.default_rng(0)
    x = rng.standard_normal((B, L, DIM), dtype=np.float32)
    import reference
    inputs = reference.setup_inputs()
    out = kernel(**{k: np.asarray(v) for k, v in inputs.items()})
    ref = np.asarray(reference.reference(**inputs))
    err = np.abs(out - ref)
    rel = np.sqrt((err ** 2).mean()) / np.sqrt((ref ** 2).mean())
    print("rms rel:", rel, "max abs:", err.max())
